# revision 83
# baseline (speedup 1.0000x reference)
"""Trainium2 Bass kernel for nn_BBoxDecoder (HyperNetwork -> per-sample CoordinateNet).

Computation:
    h1   = relu(z @ W1.T + b1)            (32, 512)
    h2   = relu(h1 @ W2.T + b2)           (32, 1024)
    flat = h2 @ W3.T + b3                 (32, 198916)   <- 815 MB of W3, the bottleneck
    per-sample 5-layer CoordinateNet on timestamps -> (32, 512, 4)

The harness gate is rel_err < 2e-2, so fp32-exact arithmetic is wasted margin.
This version streams W3 as a SINGLE fp16 plane (2 B/elem instead of the 4 B
hi/lo pair) -- halving both the HBM stream (51.4 MB/core) and the PE passes --
and runs the hypernetwork h2 and the exchanged flat params in fp16 as well.
Measured numerically, the end-to-end error of this scheme is ~7.3e-3.

Distribution over 8 NeuronCores:
  - W1 is sharded 8 ways on the output dim (64 rows/core); h1 shards are
    AllGather'd (one cheap collective at startup, saves 7.4 MB/core of DMA).
  - W3 is sharded column-wise (param dim) 8 ways, streamed once per core in
    49 chunks of 512 params; flat shards are exchanged with 3 AllToAll
    collectives whose boundaries align with CoordinateNet layer boundaries
    (collectives cost ~15us constant each in the runtime, so few+large wins;
    3 groups lets input/h0 run during group 1's stream and h1 during group
    2's stream).
  - The CoordinateNet application is data-parallel over the batch (4/core).

Precision plan (validated vs the fp64/np reference, final rel err 7.3e-3):
  z, W1, W2, h1 fp32; h2 -> fp16 single plane (x32 scaled); W3 fp16 single
  plane (x32); flat transits the AllToAll as fp16; CoordinateNet weights are
  the fp16 transit values, activations split hi/lo fp16 for the input and
  first two hidden layers and single fp16 for the last hidden + output layer
  (keeps the post-stream tail short); accumulation fp32 in PSUM throughout.
"""

import os
import sys

import numpy as np

if os.path.isdir("/opt/trn_rl_repo") and "/opt/trn_rl_repo" not in sys.path:
    sys.path.insert(0, "/opt/trn_rl_repo")

import concourse.bass as bass
import concourse.mybir as mybir
import concourse.tile as tile
from concourse.bass import ts
from concourse.bass_utils import run_bass_kernel_spmd

# ---------------------------------------------------------------- constants
B = 32          # batch
NPTS = 512      # timestamps per sample
LAT = 4096      # latent dim
H1 = 512        # hyper hidden 1
H2 = 1024       # hyper hidden 2
HID = 256       # CoordinateNet hidden dim
P_TOTAL = 198916

NCORES = 8
CH = 512                  # matmul free-dim chunk
NCH = 49                  # chunks per core (49*512 = 25088)
S = NCH * CH              # per-core shard of the padded param dim
P_PAD = NCORES * S        # 200704
BPC = B // NCORES         # 4 samples per core
H1S = H1 // NCORES        # 64 h1 rows per core

# groups of chunks; boundaries align with CoordinateNet layer param ends
# (a layer's params may span groups -- extraction handles the splits):
#   g0+g1 global 69632 >= 66304 (input+hidden0), g2 mid-hidden1,
#   g3 global 135168 >= 132096 (hidden1 complete), g4 global 167936 >=
#   166144 (hidden2 t=0 rows ..134), g5 the hidden2 tail + bias.
# Six small AllToAlls pack back-to-back on the collective cores (15us
# constant + bytes each) starting as soon as chunk 8's flat lands; the
# final, tail-latency one stays cheap, and hidden2's t=0 K-half
# accumulates while it is in flight.
CPGS = [8, 9, 8, 8, 8, 8]
NG = len(CPGS)
GWS = [c * CH for c in CPGS]                    # per-core columns in group g
LSTART = [sum(GWS[:g]) for g in range(NG)]      # local col offset of group g
GBS = [NCORES * w for w in GWS]                 # global params per group
GSTART = [sum(GBS[:g]) for g in range(NG)]      # global offset of group g

# param-space layout (extraction order, differs from torch order): the
# output-layer params are relocated BEFORE hidden2 so they arrive with the
# mid-stream group g1 instead of the final tail AllToAll:
#   [0,512) in | [512,66304) h0 | [66304,132096) h1 | [132096,133120) out.w
#   | [133120,133124) out.b | pad | [133376,198912) h2.w | [198912,199168)
#   h2.b | pad to 200704
WH_OFF = [512, 66304, 133376]
WO_OFF = 132096
BO_OFF = 133120

SCALE = 32.0              # h2 and W3 pre-scale; flat comes out x1024

FP = mybir.dt.float32
F16 = mybir.dt.float16
AF = mybir.ActivationFunctionType


# ------------------------------------------------------------- wait splitter
def _split_multi_waits(nc):
    """The walrus build here accepts at most one sync-wait per instruction.
    Engines execute in order, so hoisting all but the last wait onto fresh
    NOPs immediately before the instruction is semantically identical."""
    ctr = 0
    for f in nc.m.functions:
        for bb in f.blocks:
            out = []
            changed = False
            for ins in bb.instructions:
                si = getattr(ins, "sync_info", None)
                waits = list(si.on_wait) if (si is not None and si.on_wait) else []
                if len(waits) > 1:
                    changed = True
                    for w in waits[:-1]:
                        ctr += 1
                        out.append(
                            mybir.InstNoOp(
                                name=f"{ins.name}-sw{ctr}",
                                engine=ins.engine,
                                sync_info=mybir.SyncInfo(on_wait=[w], on_update=[]),
                            )
                        )
                    ins.sync_info = mybir.SyncInfo(
                        on_wait=waits[-1:], on_update=list(si.on_update or [])
                    )
                out.append(ins)
            if changed:
                try:
                    bb.instructions = out
                except Exception:
                    bb.instructions.clear()
                    bb.instructions.extend(out)


# ------------------------------------------------------------ device program
def _build_module(repeat: int = 1):
    nc = bass.Bass(num_devices=NCORES)

    # z and the W1 shard arrive pre-tiled to the SBUF layout: contiguous
    # 4-8KB per partition, so the startup DMAs run at full bus width
    zt_d = nc.dram_tensor("zt", [128, LAT // 128, B], FP, kind="ExternalInput")
    w1ts_d = nc.dram_tensor("w1ts", [128, LAT // 128, H1S], FP, kind="ExternalInput")
    b1s_d = nc.dram_tensor("b1s", [H1S], FP, kind="ExternalInput")
    w2t_d = nc.dram_tensor("w2t", [H1, H2], FP, kind="ExternalInput")
    b2_d = nc.dram_tensor("b2s", [H2], FP, kind="ExternalInput")     # 32*b2
    w3s_d = nc.dram_tensor("w3s", [H2 + 1, S], F16, kind="ExternalInput")
    ts_d = nc.dram_tensor("tst", [BPC, NPTS], FP, kind="ExternalInput")
    out_d = nc.dram_tensor("out", [BPC, NPTS, 4], FP, kind="ExternalOutput")

    with tile.TileContext(nc) as tc:
        with (
            tc.tile_pool(name="const", bufs=1) as const,
            tc.tile_pool(name="w3p", bufs=3) as w3p,
            tc.tile_pool(name="b3p", bufs=2) as b3p,
            tc.tile_pool(name="fsb", bufs=12) as fsb,
            tc.tile_pool(name="cpool", bufs=1) as cpool,
            tc.tile_pool(name="xpool", bufs=6) as xpool,
            tc.tile_pool(name="opool", bufs=4) as opool,
            tc.tile_pool(name="psum", bufs=8, space="PSUM") as psum,
            tc.tile_pool(name="dram", bufs=1, space="DRAM") as dram,
        ):
            for _rep in range(repeat):
                _emit_body(nc, tc, const, w3p, b3p, fsb, cpool, xpool, opool,
                           psum, dram, zt_d, w1ts_d, b1s_d, w2t_d, b2_d,
                           w3s_d, ts_d, out_d)

    _split_multi_waits(nc)
    return nc


def _emit_body(nc, tc, const, w3p, b3p, fsb, cpool, xpool, opool, psum, dram,
               zt_d, w1ts_d, b1s_d, w2t_d, b2_d, w3s_d, ts_d, out_d):
    # ---- constant loads
    w1tsb = const.tile([128, LAT // 128, H1S], FP, name="w1tsb", tag="w1tsb")
    nc.sync.dma_start(w1tsb[:], w1ts_d[:, :, :])
    zsb = const.tile([128, LAT // 128, B], FP, name="zsb", tag="zsb")
    nc.sync.dma_start(zsb[:], zt_d[:, :, :])
    b1sb = const.tile([H1S, 1], FP, name="b1sb", tag="b1sb")
    nc.sync.dma_start(b1sb[:], b1s_d[:].rearrange("(t p) -> p t", p=H1S))
    # W2/b2 go via the Pool queue: keeps them out of the AllGather's
    # straight-line DMA drain set (they are only needed for h2, after it)
    w2sb = const.tile([128, H1 // 128, H2], FP, name="w2sb", tag="w2sb")
    nc.gpsimd.dma_start(w2sb[:], w2t_d[:, :].rearrange("(t p) m -> p t m", p=128))
    b2sb = const.tile([128, H2 // 128], FP, name="b2sb", tag="b2sb")
    nc.gpsimd.dma_start(b2sb[:], b2_d[:].rearrange("(t p) -> p t", p=128))
    tssb = const.tile([1, BPC, NPTS], FP, name="tssb", tag="tssb")
    nc.sync.dma_start(tssb[:], ts_d[:, :].rearrange("(a j) n -> a j n", a=1))
    ones16 = const.tile([1, 128], F16, name="ones16", tag="ones16")
    nc.gpsimd.memset(ones16[:], 1.0)
    c32f16 = const.tile([1, B], F16, name="c32f16", tag="c32f16")
    nc.gpsimd.memset(c32f16[:], SCALE)

    # timestamps as fp16 hi/lo pair (input-layer x)
    tsh = const.tile([1, BPC, NPTS], F16, name="tsh", tag="tsh")
    tsl = const.tile([1, BPC, NPTS], F16, name="tsl", tag="tsl")
    nc.vector.tensor_copy(tsh[:], tssb[:])
    nc.vector.tensor_sub(tsl[:], tssb[:], tsh[:])

    # ---- h1 shard: rows 64c..64c+64 of h1 = relu(W1 @ z.T + b1), then
    #      AllGather so every core holds h1.T = (512, 32)
    h1ps = psum.tile([H1S, B], FP, name="h1ps", tag="ps")
    for k in range(LAT // 128):
        nc.tensor.matmul(
            h1ps[:], w1tsb[:, k, :], zsb[:, k, :],
            start=(k == 0), stop=(k == LAT // 128 - 1),
        )
    h1ssb = const.tile([H1S, B], FP, name="h1ssb", tag="h1ssb")
    nc.scalar.activation(h1ssb[:], h1ps[:], AF.Relu, bias=b1sb[:, 0:1])
    h1sh_dr = dram.tile([H1S, B], FP, name="h1sh", tag="h1sh")
    nc.gpsimd.dma_start(h1sh_dr[:, :], h1ssb[:])
    h1g_dr = dram.tile([H1, B], FP, name="h1g", tag="h1g")
    nc.gpsimd.collective_compute(
        "AllGather",
        mybir.AluOpType.bypass,
        replica_groups=[list(range(NCORES))],
        ins=[h1sh_dr.opt()],
        outs=[h1g_dr.opt()],
    )
    h1sb = const.tile([128, H1 // 128, B], FP, name="h1sb", tag="h1sb")
    nc.gpsimd.dma_start(h1sb[:], h1g_dr.rearrange("(t p) b -> p t b", p=128))

    # ---- h2 = relu(W2 @ h1 + b2), kept as 32*h2 in a single fp16 plane
    h2f = const.tile([128, H2 // 128, B], FP, name="h2f", tag="h2f")
    h2h = const.tile([128, H2 // 128, B], F16, name="h2h", tag="h2h")
    for m in range(H2 // 128):
        h2ps = psum.tile([128, B], FP, name="h2ps", tag="ps")
        for k in range(H1 // 128):
            nc.tensor.matmul(
                h2ps[:], w2sb[:, k, ts(m, 128)], h1sb[:, k, :],
                start=(k == 0), stop=(k == H1 // 128 - 1),
            )
        # 32*relu(x + b2) == relu(32x + 32*b2); b2s is pre-scaled on host
        nc.scalar.activation(
            h2f[:, m, :], h2ps[:], AF.Relu, bias=b2sb[:, m : m + 1], scale=SCALE
        )
        nc.vector.tensor_copy(h2h[:, m, :], h2f[:, m, :])

    # ---- CoordinateNet param tiles (fp16, filled from the a2a transits)
    win4 = cpool.tile([1, BPC, HID], F16, name="win4", tag="win4")
    bin4 = cpool.tile([128, BPC, 2], F16, name="bin4", tag="bin4")
    wh4s, bh4s = [], []
    for l in range(3):
        wh4s.append(cpool.tile([128, BPC, 2, HID], F16, name=f"wh4_{l}", tag=f"wh4_{l}"))
        bh4s.append(cpool.tile([128, BPC, 2], F16, name=f"bh4_{l}", tag=f"bh4_{l}"))
    wo4 = cpool.tile([128, BPC, 2, 4], F16, name="wo4", tag="wo4")
    bo4 = cpool.tile([1, BPC, 4], F16, name="bo4", tag="bo4")
    # fp32 copies of biases for ACT bias reads
    bin4f = cpool.tile([128, BPC, 2], FP, name="bin4f", tag="bin4f")
    bh4fs = [cpool.tile([128, BPC, 2], FP, name=f"bh4f_{l}", tag=f"bh4f_{l}")
             for l in range(3)]

    a2a_outs = [None] * NG

    def _extract_pieces(g, split_queues=False, all_sync=False):
        """DMA every param piece inside group g straight out of a2a_out_g.
        a2a_out rows are source-core-major: row 4s+j = sample j of my 4,
        params [GSTART[g]+s*gw + q] for local col q."""
        gw = GWS[g]
        f4g = a2a_outs[g]
        blocks = [(win4, 0, HID, HID, True)]
        blocks.append((bin4, HID, HID, 1, False))
        for l in range(3):
            a = WH_OFF[l]
            blocks.append((wh4s[l], a, HID * HID, HID, False))
            blocks.append((bh4s[l], a + HID * HID, HID, 1, False))
        blocks.append((wo4, WO_OFF, 4 * HID, 4, False))
        blocks.append((bo4, BO_OFF, 4, 4, True))
        ndma = 0

        def _piece_dma(dst, src):
            nonlocal ndma
            # in the tail both queues are idle; alternate to halve latency
            if all_sync:
                eng = nc.sync
            else:
                eng = nc.sync if (split_queues and ndma % 2) else nc.gpsimd
            eng.dma_start(dst, src)
            ndma += 1

        for dst_tile, a, length, inner, single_row in blocks:
            glo = max(a, GSTART[g])
            ghi = min(a + length, GSTART[g] + GBS[g])
            if glo >= ghi:
                continue
            for s in range(NCORES):
                clo = max(glo, GSTART[g] + s * gw)
                chi = min(ghi, GSTART[g] + (s + 1) * gw)
                if clo >= chi:
                    continue
                q0 = clo - (GSTART[g] + s * gw)
                if single_row:
                    src = f4g[4 * s : 4 * s + BPC, q0 : q0 + (chi - clo)].rearrange(
                        "(a j) o -> a j o", a=1
                    )
                    _piece_dma(dst_tile[0:1, :, clo - a : chi - a], src)
                    continue
                i0 = (clo - a) // inner
                i1 = (chi - a) // inner
                for t in range(2):
                    pa = max(i0, 128 * t)
                    pb = min(i1, 128 * (t + 1))
                    if pa >= pb:
                        continue
                    qa = q0 + (a + pa * inner - clo)
                    src = f4g[
                        4 * s : 4 * s + BPC, qa : qa + (pb - pa) * inner
                    ].rearrange("j (p o) -> p j o", o=inner)
                    if inner == 1:
                        dst = dst_tile[pa - 128 * t : pb - 128 * t, :, t : t + 1]
                    else:
                        dst = dst_tile[pa - 128 * t : pb - 128 * t, :, t, :]
                    _piece_dma(dst, src)

    xs = [None] * BPC

    def _input_layer():
        nc.vector.tensor_copy(bin4f[:], bin4[:])
        for j in range(BPC):
            xc = xpool.tile([128, 2, NPTS], FP, name="xt", tag="xt")
            for t in range(2):
                xps = psum.tile([128, NPTS], FP, name="xps", tag="ps")
                nc.tensor.matmul(
                    xps[:], win4[0:1, j, ts(t, 128)], tsh[0:1, j, :],
                    start=True, stop=False,
                )
                nc.tensor.matmul(
                    xps[:], win4[0:1, j, ts(t, 128)], tsl[0:1, j, :],
                    start=False, stop=True,
                )
                nc.scalar.activation(
                    xc[:, t, :], xps[:], AF.Relu, bias=bin4f[:, j, t : t + 1]
                )
            xs[j] = xc

    def _split_x(lo_pass, bufs=2):
        """fp16 planes of the current xs (hi only, or hi+lo)."""
        planes = []
        for j in range(BPC):
            xh = xpool.tile([128, 2, NPTS], F16, name="xh", tag="xh", bufs=bufs)
            nc.vector.tensor_copy(xh[:], xs[j][:])
            xl = None
            if lo_pass:
                xl = xpool.tile([128, 2, NPTS], F16, name="xl", tag="xl", bufs=bufs)
                nc.vector.tensor_sub(xl[:], xs[j][:], xh[:])
            planes.append((xh, xl))
        return planes

    def _hidden_layer(l, planes=None):
        # weights are the fp16 transit values; x split hi/lo for l<2,
        # single fp16 plane for the last hidden layer (l==2).
        lo_pass = l < 2
        if planes is None:
            planes = _split_x(lo_pass)
        nc.vector.tensor_copy(bh4fs[l][:], bh4s[l][:])
        for j in range(BPC):
            xh, xl = planes[j]
            # the last hidden layer writes fp16 directly: the output layer
            # consumes a single fp16 plane, so skip the fp32 round trip
            xn = xpool.tile([128, 2, NPTS], F16 if l == 2 else FP,
                            name="xt", tag="x3h" if l == 2 else "xt")
            for m in range(2):
                hps = psum.tile([128, NPTS], FP, name="hps", tag="ps")
                for t in range(2):
                    nc.tensor.matmul(
                        hps[:], wh4s[l][:, j, t, ts(m, 128)], xh[:, t, :],
                        start=(t == 0), stop=(t == 1) and not lo_pass,
                    )
                    if lo_pass:
                        nc.tensor.matmul(
                            hps[:], wh4s[l][:, j, t, ts(m, 128)], xl[:, t, :],
                            start=False, stop=(t == 1),
                        )
                nc.scalar.activation(
                    xn[:, m, :], hps[:], AF.Relu, bias=bh4fs[l][:, j, m : m + 1]
                )
            xs[j] = xn

    # hidden layer 2 split around the last AllToAll: weight rows 0..134
    # arrive with groups 1+2 (covering the whole t=0 K-tile), the rest
    # (+bias) only with group 3 -- the t=0 half accumulates during that a2a
    h2ps_held = []

    def _h2_start(planes):
        for j in range(BPC):
            xh, _ = planes[j]
            for m in range(2):
                hps = psum.tile([128, NPTS], FP, name="hps", tag="ps")
                nc.tensor.matmul(
                    hps[:], wh4s[2][:, j, 0, ts(m, 128)], xh[:, 0, :],
                    start=True, stop=False,
                )
                h2ps_held.append(hps)

    def _h2_finish(planes):
        nc.vector.tensor_copy(bh4fs[2][:], bh4s[2][:])
        for j in range(BPC):
            xh, _ = planes[j]
            xn = xpool.tile([128, 2, NPTS], F16, name="x3h", tag="x3h")
            for m in range(2):
                hps = h2ps_held[j * 2 + m]
                nc.tensor.matmul(
                    hps[:], wh4s[2][:, j, 1, ts(m, 128)], xh[:, 1, :],
                    start=False, stop=True,
                )
                nc.scalar.activation(
                    xn[:, m, :], hps[:], AF.Relu, bias=bh4fs[2][:, j, m : m + 1]
                )
            xs[j] = xn

    def _output_layer():
        for j in range(BPC):
            xh = xs[j]                       # fp16 plane from hidden layer 2
            outj = opool.tile([128, 4, 4], FP, name="outj", tag="outj")
            for m in range(4):
                ops_ = psum.tile([128, 4], FP, name="ops", tag="ps")
                for t in range(2):
                    nc.tensor.matmul(
                        ops_[:], xh[:, t, ts(m, 128)], wo4[:, j, t, :],
                        start=(t == 0), stop=False,
                    )
                nc.tensor.matmul(
                    ops_[:], ones16[:, :128], bo4[0:1, j, :], start=False, stop=True
                )
                nc.scalar.activation(outj[:, m, :], ops_[:], AF.Sigmoid)
            eng = nc.sync if j % 2 else nc.gpsimd
            eng.dma_start(
                out_d[j, :, :].rearrange("(m p) o -> p m o", p=128), outj[:]
            )

    # ---- the W3 stream: 49 chunks in 3 groups, pipelined AllToAll exchange.
    # Per-engine program order matters: extract/layer work for group g is
    # emitted right after group g+1's chunks so the in-order PE stream never
    # stalls waiting for a collective.
    a2a_ins = []
    for g in range(NG):
        a2a_ins.append(dram.tile([B, GWS[g]], F16, name=f"a2ain{g}", tag=f"a2ain{g}"))
        a2a_outs[g] = dram.tile([B, GWS[g]], F16, name=f"a2aout{g}", tag=f"a2aout{g}")

    b3rs = {}

    def _load_b3(g):
        b3r = b3p.tile([1, GWS[g]], F16, name=f"b3r{g}", tag="b3r")
        nc.gpsimd.dma_start(
            b3r[:], w3s_d[H2 : H2 + 1, LSTART[g] : LSTART[g] + GWS[g]]
        )
        b3rs[g] = b3r

    def _stream_chunks(g, lo, hi, w3c_eng=None):
        b3r = b3rs[g]
        for cc in range(lo, hi):
            c0 = LSTART[g] + cc * CH
            w3c = w3p.tile([128, H2 // 128, CH], F16, name="w3c", tag="w3c")
            (w3c_eng or nc.sync).dma_start(
                w3c[:],
                w3s_d[0:H2, c0 : c0 + CH].rearrange("(t p) c -> p t c", p=128),
            )
            fps = psum.tile([B, CH], FP, name="fps", tag="ps")
            for k in range(H2 // 128):
                nc.tensor.matmul(
                    fps[:], h2h[:, k, :], w3c[:, k, :],
                    start=(k == 0), stop=False,
                )
            nc.tensor.matmul(
                fps[:], c32f16[:], b3r[0:1, ts(cc, CH)], start=False, stop=True
            )
            # undo the 32*32 pre-scale (exact power of two), emit fp16 transit
            fsb_t = fsb.tile([B, CH], F16, name="fsb", tag="fsb")
            nc.scalar.mul(fsb_t[:], fps[:], 1.0 / 1024.0)
            nc.gpsimd.dma_start(a2a_ins[g][:, cc * CH : (cc + 1) * CH], fsb_t[:])

    def _issue_a2a(g):
        nc.gpsimd.collective_compute(
            "AllToAll",
            mybir.AluOpType.bypass,
            replica_groups=[list(range(NCORES))],
            ins=[a2a_ins[g].opt()],
            outs=[a2a_outs[g].opt()],
        )

    # tile_wait_until: virtual-time floors that pin the SCHEDULED order --
    # without them the list scheduler hoists ready DMAs ahead of the
    # collectives, which then stall on the straight-line DMA-drain rule
    # (a collective waits for every DMA issued before it in final order).
    with tc.tile_wait_until(0.04):
        _load_b3(0)                # keep the first W3 prefetches out of the
        _load_b3(1)                # AllGather's drain set: on the Pool
        # queue they sit AFTER the AllGather, so its straight-line drain
        # only covers the small constant loads
        _stream_chunks(0, 0, 3)
    _stream_chunks(0, 3, 8)
    _issue_a2a(0)
    _load_b3(2)
    _stream_chunks(1, 0, 9)
    _issue_a2a(1)
    _load_b3(3)
    _stream_chunks(2, 0, 8)
    _issue_a2a(2)
    _load_b3(4)
    _stream_chunks(3, 0, 5)
    _extract_pieces(0)             # one extract batch per mid-group Pool
    _stream_chunks(3, 5, 8)        # slot: each a2a is long done by its
    _issue_a2a(3)                  # slot, and the halved holds no longer
    _load_b3(5)                    # push the next a2a past its data
    _stream_chunks(4, 0, 5)
    _extract_pieces(1)
    _stream_chunks(4, 5, 8)
    _issue_a2a(4)
    _stream_chunks(5, 0, 8)
    _issue_a2a(5)
    _input_layer()                 # PE is DMA-gated through g5's chunks;
    _hidden_layer(0)               # these fill that idle window
    with tc.tile_wait_until(0.2):
        _extract_pieces(2)                  # after a2a5 in final order, so
        _extract_pieces(3, all_sync=True)   # no a2a drain includes them
        _extract_pieces(4)
    _hidden_layer(1)
    planes2 = _split_x(False)
    _h2_start(planes2)             # hidden2 K-slab during the last a2a
    with tc.tile_wait_until(0.21):
        _extract_pieces(5, split_queues=True)
    _h2_finish(planes2)
    _output_layer()


_NC_CACHE = {}


def _get_module(repeat: int = 1):
    if repeat not in _NC_CACHE:
        _NC_CACHE[repeat] = _build_module(repeat)
    return _NC_CACHE[repeat]


# -------------------------------------------------------------- host wrapper
def _build_perm():
    """src[i] = torch-order index of padded-layout position i (-1 = pad)."""
    g = np.arange(HID * HID, dtype=np.int64).reshape(HID, HID)
    whT = g.T.ravel()
    src = np.full(P_PAD, -1, dtype=np.int64)
    src[0:512] = np.arange(512)
    for l in range(2):                       # h0, h1: same offsets as torch
        a = 512 + l * (HID * HID + HID)
        src[a : a + HID * HID] = a + whT
        src[a + HID * HID : a + HID * HID + HID] = np.arange(
            a + HID * HID, a + HID * HID + HID
        )
    g2 = np.arange(4 * HID, dtype=np.int64).reshape(4, HID)
    src[WO_OFF : WO_OFF + 4 * HID] = 197888 + g2.T.ravel()
    src[BO_OFF : BO_OFF + 4] = np.arange(198912, 198916)
    src[WH_OFF[2] : WH_OFF[2] + HID * HID] = 132096 + whT
    src[WH_OFF[2] + HID * HID : WH_OFF[2] + HID * HID + HID] = np.arange(
        197632, 197888
    )
    return src


_PERM_CACHE = None
LAST_RESULTS = None


def prepare_in_maps(z, timestamps, W1, b1, W2, b2, W3, b3):
    global _PERM_CACHE
    z = np.asarray(z, np.float32)
    timestamps = np.asarray(timestamps, np.float32)
    W1 = np.asarray(W1, np.float32)
    b1 = np.asarray(b1, np.float32)
    W2 = np.asarray(W2, np.float32)
    b2 = np.asarray(b2, np.float32)
    W3 = np.asarray(W3, np.float32)
    b3 = np.asarray(b3, np.float32)

    if _PERM_CACHE is None:
        _PERM_CACHE = _build_perm()
    src = _PERM_CACHE

    # pre-tile z.T / W1.T-shards to [128, LAT//128, cols] SBUF layout
    zt = np.ascontiguousarray(
        z.T.reshape(LAT // 128, 128, B).transpose(1, 0, 2)
    )
    w1t = np.ascontiguousarray(W1.T)
    w2t = np.ascontiguousarray(W2.T)
    b2s = SCALE * b2
    mask = src >= 0
    Wp_pad = np.zeros((P_PAD, H2), np.float32)
    Wp_pad[mask] = W3[src[mask]]             # rows in extraction order
    bp_pad = np.zeros((P_PAD,), np.float32)
    bp_pad[mask] = b3[src[mask]]

    in_maps = []
    for c in range(NCORES):
        w3s_c = np.zeros((H2 + 1, S), np.float16)
        for g in range(NG):
            glo = GSTART[g] + c * GWS[g]
            ws = SCALE * Wp_pad[glo : glo + GWS[g]]              # (gw, 1024)
            cs = slice(LSTART[g], LSTART[g] + GWS[g])
            w3s_c[:H2, cs] = ws.astype(np.float16).T
            # bias row: psum accumulates 32*(32*b3) = 1024*b3
            w3s_c[H2, cs] = (SCALE * bp_pad[glo : glo + GWS[g]]).astype(np.float16)
        in_maps.append(
            {
                "zt": zt,
                "w1ts": np.ascontiguousarray(
                    w1t[:, c * H1S : (c + 1) * H1S]
                    .reshape(LAT // 128, 128, H1S)
                    .transpose(1, 0, 2)
                ),
                "b1s": np.ascontiguousarray(b1[c * H1S : (c + 1) * H1S]),
                "w2t": w2t,
                "b2s": b2s,
                "w3s": w3s_c,
                "tst": np.ascontiguousarray(
                    timestamps[c * BPC : (c + 1) * BPC, :, 0]
                ),
            }
        )
    return in_maps


def kernel(z, timestamps, W1, b1, W2, b2, W3, b3):
    global LAST_RESULTS
    in_maps = prepare_in_maps(z, timestamps, W1, b1, W2, b2, W3, b3)
    nc = _get_module(1)
    res = run_bass_kernel_spmd(nc, in_maps, core_ids=list(range(NCORES)))
    LAST_RESULTS = res
    out = np.concatenate(
        [np.asarray(res.results[c]["out"]) for c in range(NCORES)], axis=0
    )
    return out.astype(np.float32, copy=False)
